# revision 20
# baseline (speedup 1.0000x reference)
"""AttentionBlock Trainium2 kernel (nn_AttentionBlock_74010876445388).

Strategy: data-parallel over batch (B=16 -> 2 per core x 8 cores).
Per core, for each of its 2 images:
  - GroupNorm(32 groups) via bn_stats + PE selector-matmul group reduction
    (stats kept in f32/f32r for precision)
  - qkv projection in bf16 (weights pre-cast host-side, DMA'd as bf16)
  - attention in S^T layout: S^T = k^T q with the two heads of a pair
    issued back-to-back on row-groups (0,0)/(64,0) so they stream
    concurrently through the PE array; one exp per sj over the merged
    [128,2048] PSUM; PV in bf16 with v^T carrying a ones column that
    accumulates the softmax denominator for free
  - denominator: ACT copy psum-row -> p0, DVE reciprocal_approx_fast,
    broadcast via K=1 ones-matmul
  - output projection + bias (v-bias folded host-side into b_eff)
bf16 operands keep rel err ~5e-3 (<< 2e-2 gate) while halving PE
streaming cycles vs f32r and enabling fast weight loads + 2x DVE modes.
"""

import numpy as np

import concourse.bass as bass
import concourse.tile as tile
from concourse import bacc, mybir
from concourse.bass_utils import run_bass_kernel_spmd

N_CORES = 8
B, C, HW_L = 16, 512, 1024  # full batch, channels, flattened spatial
BPC = B // N_CORES  # batches per core = 2
NH = 8  # heads
CH = C // NH  # 64 channels/head
NG = 32  # groups
GS = C // NG  # 16 channels/group
L = HW_L
EPS = 1e-5
F32 = mybir.dt.float32
F32R = mybir.dt.float32r
BF16 = mybir.dt.bfloat16
AF = mybir.ActivationFunctionType
OP = mybir.AluOpType

_nc_cache = None


def _build(debug=False):
    nc = bacc.Bacc("TRN2", target_bir_lowering=False)

    x2 = nc.dram_tensor("x2", [BPC, C, L], F32, kind="ExternalInput")
    wqkT = nc.dram_tensor("wqkT", [C, 3 * C], BF16, kind="ExternalInput")
    wpT = nc.dram_tensor("wpT", [C, C], BF16, kind="ExternalInput")
    # packed per-partition constants: cols = bq[0:4] bk[4:8] beff[8:12]
    # nw[12:16] nb[16:20]
    cvec = nc.dram_tensor("cvec", [128, 20], F32, kind="ExternalInput")
    sel = nc.dram_tensor("sel", [128, 4 * NG], F32, kind="ExternalInput")
    esel = nc.dram_tensor("esel", [NG, 4 * 128], F32, kind="ExternalInput")
    out_d = nc.dram_tensor("out", [BPC, C, L], F32, kind="ExternalOutput")

    from contextlib import ExitStack

    with tile.TileContext(nc) as tc, ExitStack() as es:
        cst_pool = es.enter_context(tc.tile_pool(name="const", bufs=1))
        wstage = es.enter_context(tc.tile_pool(name="wstage", bufs=1))
        xb_pool = es.enter_context(tc.tile_pool(name="xb", bufs=2))
        h_pool = es.enter_context(tc.tile_pool(name="hb", bufs=2))
        qk_pool = es.enter_context(tc.tile_pool(name="qk", bufs=4))
        vt_pool = es.enter_context(tc.tile_pool(name="vt", bufs=1))
        e_pool = es.enter_context(tc.tile_pool(name="ee", bufs=6))
        a_pool = es.enter_context(tc.tile_pool(name="ab", bufs=1))
        sm_pool = es.enter_context(tc.tile_pool(name="sm", bufs=2))
        rc_pool = es.enter_context(tc.tile_pool(name="rc", bufs=2))
        au_pool = es.enter_context(tc.tile_pool(name="au", bufs=2))
        o_pool = es.enter_context(tc.tile_pool(name="ob", bufs=2))
        ps_s0 = es.enter_context(tc.tile_pool(name="ps_s0", bufs=1, space="PSUM"))
        ps_s1 = es.enter_context(tc.tile_pool(name="ps_s1", bufs=1, space="PSUM"))
        ps_aA = es.enter_context(tc.tile_pool(name="ps_aA", bufs=1, space="PSUM"))
        ps_aB = es.enter_context(tc.tile_pool(name="ps_aB", bufs=1, space="PSUM"))

        # ---- image-0 x load first (critical path), then weights ----
        x_pre = []
        for j in range(4):
            x_t = xb_pool.tile([128, L], F32, tag=f"x{j}")
            nc.gpsimd.dma_start(x_t, x2[0, 128 * j : 128 * (j + 1), :])
            x_pre.append(x_t)
        # weights arrive pre-cast bf16 from host
        wq_sb = []
        for j in range(4):
            wt = cst_pool.tile([128, 3 * C], BF16, tag=f"wq{j}")
            nc.sync.dma_start(wt, wqkT[128 * j : 128 * (j + 1), :])
            wq_sb.append(wt)
        wp_sb = []
        for j in range(4):
            wt = cst_pool.tile([128, C], BF16, tag=f"wp{j}")
            nc.sync.dma_start(wt, wpT[128 * j : 128 * (j + 1), :])
            wp_sb.append(wt)
        # groupnorm selectors stay f32r for stats precision
        st = wstage.tile([128, 4 * NG], F32, tag="selst")
        nc.sync.dma_start(st, sel.ap())
        sel_sb = cst_pool.tile([128, 4 * NG], F32R, tag="sel")
        nc.vector.tensor_copy(sel_sb, st)
        st = wstage.tile([NG, 4 * 128], F32, tag="eselst")
        nc.sync.dma_start(st, esel.ap())
        esel_sb = cst_pool.tile([NG, 4 * 128], F32R, tag="esel")
        nc.vector.tensor_copy(esel_sb, st)
        cv = cst_pool.tile([128, 20], F32, tag="cvec")
        nc.sync.dma_start(cv, cvec.ap())
        bq_sb = [cv[:, i : i + 1] for i in range(0, 4)]
        bk_sb = [cv[:, i : i + 1] for i in range(4, 8)]
        beff_sb = [cv[:, i : i + 1] for i in range(8, 12)]
        nw_sb = [cv[:, i : i + 1] for i in range(12, 16)]
        nb_sb = [cv[:, i : i + 1] for i in range(16, 20)]
        ones_f = cst_pool.tile([128, 1], F32, tag="ones_f")
        nc.vector.memset(ones_f, 1.0)
        ones_b = cst_pool.tile([128, 1], BF16, tag="ones_b")
        nc.vector.memset(ones_b, 1.0)
        ones1r = cst_pool.tile([1, 64], BF16, tag="ones1r")
        nc.vector.tensor_copy(ones1r, ones_f[0:1, :].to_broadcast((1, 64)))

        # contiguous slices into the host-reordered weight:
        # cols [0:512]=q pair-major, [512:1024]=k pair-major,
        # [1024:1536]=v head-major
        def wq_ap(j, p):
            return wq_sb[j][:, 128 * p : 128 * (p + 1)]

        def wk_ap(j, p):
            return wq_sb[j][:, 512 + 128 * p : 512 + 128 * (p + 1)]

        def wv_ap(j, g):
            return wq_sb[j][:, 1024 + 256 * g : 1024 + 256 * (g + 1)]

        def emit_proj(bb, ab_tiles):
            for m in range(4):
                pj_pool, pj_tag = ((ps_aA, "aA"), (ps_aB, "aB"))[m % 2]
                pj = pj_pool.tile([128, 1024], F32, tag=pj_tag)
                for n in range(2):
                    for j in range(4):
                        nc.tensor.matmul(
                            pj[:, 512 * n : 512 * (n + 1)],
                            wp_sb[j][:, 128 * m : 128 * (m + 1)],
                            ab_tiles[j][:, 512 * n : 512 * (n + 1)],
                            start=(j == 0),
                            stop=(j == 3),
                        )
                o_t = o_pool.tile([128, L], F32, tag="o")
                nc.vector.tensor_scalar_add(o_t, pj, beff_sb[m])
                nc.sync.dma_start(out_d[bb, 128 * m : 128 * (m + 1), :], o_t)

        for b in range(BPC):
            # ---- load x + per-channel stats ----
            xt = []
            mv_t = []
            for j in range(4):
                if b == 0:
                    x_t = x_pre[j]
                else:
                    x_t = xb_pool.tile([128, L], F32, tag=f"x{j}")
                    nc.gpsimd.dma_start(x_t, x2[b, 128 * j : 128 * (j + 1), :])
                xt.append(x_t)
                stats = sm_pool.tile([128, 2, 6], F32, tag="bnst")
                nc.vector.bn_stats(stats[:, 0, :], x_t[:, 0:512])
                nc.vector.bn_stats(stats[:, 1, :], x_t[:, 512:1024])
                mv = sm_pool.tile([128, 2], F32, tag=f"mv{j}")
                nc.vector.bn_aggr(mv, stats)
                mv_t.append(mv)

            # ss = [mean, E[x^2]] per channel, rounded to f32r
            gps = ps_s0.tile([128, 2], F32, tag="s0")
            for j in range(4):
                ss = sm_pool.tile([128, 2], F32R, tag=f"ss{j}")
                msq = sm_pool.tile([128, 1], F32, tag=f"msq{j}")
                nc.vector.tensor_copy(ss[:, 0:1], mv_t[j][:, 0:1])
                nc.vector.tensor_tensor(
                    msq, mv_t[j][:, 0:1], mv_t[j][:, 0:1], OP.mult
                )
                nc.vector.tensor_tensor(ss[:, 1:2], mv_t[j][:, 1:2], msq, OP.add)
                nc.tensor.matmul(
                    gps[0:NG, 0:2],
                    sel_sb[:, NG * j : NG * (j + 1)],
                    ss,
                    start=(j == 0),
                    stop=(j == 3),
                )

            # group stats -> [mean, rstd]
            gsb = sm_pool.tile([NG, 2], F32, tag="gsb")
            nc.vector.tensor_copy(gsb, gps[0:NG, 0:2])
            gm2 = sm_pool.tile([NG, 1], F32, tag="gm2")
            nc.vector.tensor_tensor(gm2, gsb[:, 0:1], gsb[:, 0:1], OP.mult)
            gvar = sm_pool.tile([NG, 1], F32, tag="gvar")
            nc.vector.tensor_tensor(gvar, gsb[:, 1:2], gm2, OP.subtract)
            nc.vector.tensor_scalar_add(gvar, gvar, EPS)
            # rsqrt via bit trick + 2 Newton iterations (avoids ACT table swap)
            y_i = sm_pool.tile([NG, 1], mybir.dt.int32, tag="rs_i")
            nc.vector.tensor_scalar(
                y_i, gvar.bitcast(mybir.dt.int32), 1, None, OP.logical_shift_right
            )
            nc.vector.tensor_scalar(y_i, y_i, -1, 0x5F3759DF, OP.mult, OP.add)
            y = y_i.bitcast(F32)
            gstats = sm_pool.tile([NG, 2], F32R, tag="gst")
            nc.vector.tensor_copy(gstats[:, 0:1], gsb[:, 0:1])
            tmp = sm_pool.tile([NG, 1], F32, tag="rs_t")
            for _ in range(2):
                nc.vector.tensor_tensor(tmp, y, y, OP.mult)
                nc.vector.tensor_tensor(tmp, tmp, gvar, OP.mult)
                nc.vector.tensor_scalar(tmp, tmp, -0.5, 1.5, OP.mult, OP.add)
                nc.vector.tensor_tensor(y, y, tmp, OP.mult)
            nc.vector.tensor_copy(gstats[:, 1:2], y)

            # expand per-group -> per-channel, normalize -> h (bf16)
            ht = []
            for j in range(4):
                cst_ps = ps_s1.tile([128, 2], F32, tag="s1")
                nc.tensor.matmul(
                    cst_ps[:, 0:2],
                    esel_sb[:, 128 * j : 128 * (j + 1)],
                    gstats,
                    start=True,
                    stop=True,
                )
                sc = sm_pool.tile([128, 1], F32, tag=f"sc{j}")
                nc.vector.tensor_tensor(sc, cst_ps[:, 1:2], nw_sb[j], OP.mult)
                bi = sm_pool.tile([128, 1], F32, tag=f"bi{j}")
                nc.vector.tensor_tensor(bi, cst_ps[:, 0:1], sc, OP.mult)
                nc.vector.tensor_tensor(bi, nb_sb[j], bi, OP.subtract)
                h_t = h_pool.tile([128, L], BF16, tag=f"h{j}")
                nc.vector.tensor_scalar(h_t, xt[j], sc, bi, OP.mult, OP.add)
                ht.append(h_t)

            # ---- deferred proj of previous image (fills PE idle here) ----
            if b > 0:
                emit_proj(b - 1, prev_a_sb)

            # ---- v^T production: per head 64 v-cols + a ones column
            # (the ones column makes PV accumulate the softmax
            # denominator in psum row 64 for free)
            vt_sb = vt_pool.tile([128, 8, 8, 65], BF16, tag="vt")
            nc.vector.tensor_copy(
                vt_sb[:, :, :, 64:65],
                ones_b[:, None, None, :].to_broadcast((128, 8, 8, 1)),
            )
            for i in range(8):
                vpool, vtag = ((ps_s0, "s0"), (ps_s1, "s1"))[i % 2]
                vps = vpool.tile([128, 512], F32, tag=vtag)
                for j in range(4):
                    nc.tensor.matmul(
                        vps,
                        ht[j][:, 128 * i : 128 * (i + 1)],
                        wq_sb[j][:, 1024:1536],
                        start=(j == 0),
                        stop=(j == 3),
                    )
                nc.vector.tensor_copy(
                    vt_sb[:, i, :, 0:64],
                    vps[:, 0:512].rearrange("p (h c) -> p h c", c=64),
                )

            # ---- qkv for all 4 pairs (hoisted; overlaps freely) ----
            qk_t = []
            for p in range(4):
                qps = ps_s0.tile([128, 1024], F32, tag="s0")
                kps = ps_s1.tile([128, 1024], F32, tag="s1")
                for n in range(2):
                    for j in range(4):
                        nc.tensor.matmul(
                            qps[:, 512 * n : 512 * (n + 1)],
                            wq_ap(j, p),
                            ht[j][:, 512 * n : 512 * (n + 1)],
                            start=(j == 0),
                            stop=(j == 3),
                        )
                        nc.tensor.matmul(
                            kps[:, 512 * n : 512 * (n + 1)],
                            wk_ap(j, p),
                            ht[j][:, 512 * n : 512 * (n + 1)],
                            start=(j == 0),
                            stop=(j == 3),
                        )
                q_sb = qk_pool.tile([128, L], BF16, tag="q")
                nc.vector.tensor_scalar_add(q_sb, qps, bq_sb[p])
                k_sb = qk_pool.tile([128, L], BF16, tag="k")
                nc.vector.tensor_scalar_add(k_sb, kps, bk_sb[p])
                qk_t.append((q_sb, k_sb))

            # ---- attention per head-pair ----
            a_sb = []
            for p in range(4):
                q_sb, k_sb = qk_t[p]
                # attention: per sj, 4 S matmuls issued as row-group pairs
                # (halves stream concurrently), one exp over [128,2048],
                # then 4 PV matmuls accumulating [65,1024] per head
                # (psum row 64 = softmax denominator via ones column)
                a_psA = ps_aA.tile([128, 1024], F32, tag="aA")
                a_psB = ps_aB.tile([128, 1024], F32, tag="aB")
                for sj in range(8):
                    sl = slice(128 * sj, 128 * (sj + 1))
                    s_ps0 = ps_s0.tile([128, 1024], F32, tag="s0")
                    s_ps1 = ps_s1.tile([128, 1024], F32, tag="s1")
                    for n in range(2):
                        nc.tensor.matmul(
                            s_ps0[:, 512 * n : 512 * (n + 1)],
                            k_sb[0:64, sl],
                            q_sb[0:64, 512 * n : 512 * (n + 1)],
                            start=True,
                            stop=True,
                            tile_position=(0, 0),
                        )
                        nc.tensor.matmul(
                            s_ps1[:, 512 * n : 512 * (n + 1)],
                            k_sb[64:128, sl],
                            q_sb[64:128, 512 * n : 512 * (n + 1)],
                            start=True,
                            stop=True,
                            tile_position=(64, 0),
                        )
                    for half, (s_ps, a_ps) in enumerate(
                        ((s_ps0, a_psA), (s_ps1, a_psB))
                    ):
                        e_t = e_pool.tile([128, 1024], BF16, tag="e")
                        nc.scalar.activation(e_t, s_ps, AF.Exp, scale=0.125)
                        for n in range(2):
                            nsl = slice(512 * n, 512 * (n + 1))
                            nc.tensor.matmul(
                                a_ps[0:65, nsl],
                                vt_sb[:, sj, 2 * p + half, :],
                                e_t[:, nsl],
                                start=(sj == 0),
                                stop=(sj == 7),
                            )

                # denominators -> p0 via ACT, approx-recip, broadcast by
                # K=1 ones-matmul into freed psum, then normalize
                dsbA = rc_pool.tile([1, 1024], F32, tag="dsbA")
                nc.scalar.activation(dsbA, a_psA[64:65, :], AF.Copy)
                rawA = rc_pool.tile([1, 1024], F32, tag="rawA")
                nc.vector.reciprocal_approx_fast(rawA, dsbA)
                recipA = rc_pool.tile([1, 1024], BF16, tag="rcA")
                nc.vector.tensor_copy(recipA, rawA)
                dsbB = rc_pool.tile([1, 1024], F32, tag="dsbB")
                nc.vector.tensor_copy(dsbB, a_psB[64:65, :])
                rawB = rc_pool.tile([1, 1024], F32, tag="rawB")
                nc.vector.reciprocal_approx_fast(rawB, dsbB)
                recipB = rc_pool.tile([1, 1024], BF16, tag="rcB")
                nc.vector.tensor_copy(recipB, rawB)
                a_unA = au_pool.tile([64, 1024], BF16, tag="a_unA")
                nc.vector.tensor_copy(a_unA, a_psA[0:64, :])
                a_unB = au_pool.tile([64, 1024], BF16, tag="a_unB")
                nc.vector.tensor_copy(a_unB, a_psB[0:64, :])
                bcA_ps = ps_aA.tile([128, 1024], F32, tag="aA")
                bcB_ps = ps_aB.tile([128, 1024], F32, tag="aB")
                for n in range(2):
                    nsl = slice(512 * n, 512 * (n + 1))
                    nc.tensor.matmul(
                        bcA_ps[0:64, nsl], ones1r, recipA[:, nsl],
                        start=True, stop=True,
                    )
                    nc.tensor.matmul(
                        bcB_ps[0:64, nsl], ones1r, recipB[:, nsl],
                        start=True, stop=True,
                    )
                a_t = a_pool.tile([128, L], BF16, tag=f"a{p}")
                nc.vector.tensor_tensor(a_t[0:64, :], a_unA, bcA_ps[0:64, :], OP.mult)
                nc.vector.tensor_tensor(
                    a_t[64:128, :], a_unB, bcB_ps[0:64, :], OP.mult
                )
                a_sb.append(a_t)

            prev_a_sb = a_sb

        # final image's proj
        emit_proj(BPC - 1, prev_a_sb)

    nc.compile()
    return nc


def _get_nc():
    global _nc_cache
    if _nc_cache is None:
        _nc_cache = _build()
    return _nc_cache


def _prep_inputs(x, norm_w, norm_b, w_qkv, b_qkv, w_proj, b_proj):
    import ml_dtypes

    x = np.asarray(x, dtype=np.float32).reshape(B, C, L)
    w_qkv = np.asarray(w_qkv, dtype=np.float32)
    b_qkv = np.asarray(b_qkv, dtype=np.float32)
    w_proj = np.asarray(w_proj, dtype=np.float32)
    b_proj = np.asarray(b_proj, dtype=np.float32)
    norm_w = np.asarray(norm_w, dtype=np.float32)
    norm_b = np.asarray(norm_b, dtype=np.float32)

    # column-reordered transposed qkv weight: [C, 3C] with
    # q pair-major | k pair-major | v head-major, all contiguous
    wqkT = np.zeros((C, 3 * C), dtype=np.float32)
    wT = w_qkv.T  # [C, 3C] original row order (per head: q,k,v)
    for h in range(NH):
        base = 192 * h
        wqkT[:, 64 * h : 64 * (h + 1)] = wT[:, base : base + 64]
        wqkT[:, 512 + 64 * h : 512 + 64 * (h + 1)] = wT[:, base + 64 : base + 128]
        wqkT[:, 1024 + 64 * h : 1024 + 64 * (h + 1)] = wT[:, base + 128 : base + 192]
    wqkT = np.ascontiguousarray(wqkT.astype(ml_dtypes.bfloat16))
    wpT = np.ascontiguousarray(w_proj.T.astype(ml_dtypes.bfloat16))  # [C, C]

    # per-pair q/k biases: [pair, {q,k}, 128]
    bqk = np.zeros((4, 2, 128), dtype=np.float32)
    for p in range(4):
        for half, h in enumerate((2 * p, 2 * p + 1)):
            base = 192 * h
            bqk[p, 0, 64 * half : 64 * (half + 1)] = b_qkv[base : base + 64]
            bqk[p, 1, 64 * half : 64 * (half + 1)] = b_qkv[base + 64 : base + 128]
    del base

    # v bias folded into proj bias: b_eff = b_proj + w_proj @ bv
    bv = np.zeros((C,), dtype=np.float32)
    for h in range(NH):
        bv[64 * h : 64 * (h + 1)] = b_qkv[192 * h + 128 : 192 * h + 192]
    b_eff = (b_proj.astype(np.float64) + w_proj.astype(np.float64) @ bv).astype(
        np.float32
    )

    sel = np.zeros((128, 4 * NG), dtype=np.float32)
    esel = np.zeros((NG, 4 * 128), dtype=np.float32)
    for j in range(4):
        for c in range(128):
            sel[c, NG * j + 8 * j + c // GS] = 1.0 / GS
            esel[8 * j + c // GS, 128 * j + c] = 1.0

    cv = np.zeros((128, 20), dtype=np.float32)
    cv[:, 0:4] = bqk[:, 0, :].T
    cv[:, 4:8] = bqk[:, 1, :].T
    cv[:, 8:12] = b_eff.reshape(4, 128).T
    cv[:, 12:16] = norm_w.reshape(4, 128).T
    cv[:, 16:20] = norm_b.reshape(4, 128).T

    shared = {
        "wqkT": wqkT,
        "wpT": wpT,
        "cvec": cv,
        "sel": sel,
        "esel": esel,
    }
    in_maps = []
    for c in range(N_CORES):
        m = dict(shared)
        m["x2"] = np.ascontiguousarray(x[BPC * c : BPC * (c + 1)])
        in_maps.append(m)
    return in_maps


def _run(in_maps, trace=False):
    nc = _get_nc()
    return run_bass_kernel_spmd(
        nc, in_maps, core_ids=list(range(N_CORES)), trace=trace
    )


def kernel(x, norm_w, norm_b, w_qkv, b_qkv, w_proj, b_proj):
    in_maps = _prep_inputs(x, norm_w, norm_b, w_qkv, b_qkv, w_proj, b_proj)
    res = _run(in_maps)
    out = np.concatenate([r["out"] for r in res.results], axis=0)
    return out.astype(np.float32)


# revision 22
# speedup vs baseline: 1.1403x; 1.1403x over previous
"""AttentionBlock Trainium2 kernel (nn_AttentionBlock_74010876445388).

Strategy: data-parallel over batch (B=16 -> 2 per core x 8 cores).
Per core, for each of its 2 images:
  - GroupNorm(32 groups) via bn_stats + PE selector-matmul group reduction
    (stats kept in f32/f32r for precision)
  - qkv projection in bf16 (weights pre-cast host-side, DMA'd as bf16)
  - attention in S^T layout: S^T = k^T q with the two heads of a pair
    issued back-to-back on row-groups (0,0)/(64,0) so they stream
    concurrently through the PE array; one exp per sj over the merged
    [128,2048] PSUM; PV in bf16 with v^T carrying a ones column that
    accumulates the softmax denominator for free
  - denominator: ACT copy psum-row -> p0, DVE reciprocal_approx_fast,
    broadcast via K=1 ones-matmul
  - output projection + bias (v-bias folded host-side into b_eff)
bf16 operands keep rel err ~5e-3 (<< 2e-2 gate) while halving PE
streaming cycles vs f32r and enabling fast weight loads + 2x DVE modes.
"""

import numpy as np

import concourse.bass as bass
import concourse.tile as tile
from concourse import bacc, mybir
from concourse.bass_utils import run_bass_kernel_spmd

N_CORES = 8
B, C, HW_L = 16, 512, 1024  # full batch, channels, flattened spatial
BPC = B // N_CORES  # batches per core = 2
NH = 8  # heads
CH = C // NH  # 64 channels/head
NG = 32  # groups
GS = C // NG  # 16 channels/group
L = HW_L
EPS = 1e-5
F32 = mybir.dt.float32
F32R = mybir.dt.float32r
BF16 = mybir.dt.bfloat16
AF = mybir.ActivationFunctionType
OP = mybir.AluOpType

_nc_cache = None


def _build(debug=False):
    nc = bacc.Bacc("TRN2", target_bir_lowering=False)

    x2 = nc.dram_tensor("x2", [BPC, C, L], F32, kind="ExternalInput")
    wqkT = nc.dram_tensor("wqkT", [C, 3 * C], BF16, kind="ExternalInput")
    wpT = nc.dram_tensor("wpT", [C, C], BF16, kind="ExternalInput")
    # packed per-partition constants: cols = bq[0:4] bk[4:8] beff[8:12]
    # nw[12:16] nb[16:20]
    cvec = nc.dram_tensor("cvec", [128, 20], F32, kind="ExternalInput")
    sel = nc.dram_tensor("sel", [128, 4 * NG], F32, kind="ExternalInput")
    esel = nc.dram_tensor("esel", [NG, 4 * 128], F32, kind="ExternalInput")
    out_d = nc.dram_tensor("out", [BPC, C, L], F32, kind="ExternalOutput")

    from contextlib import ExitStack

    with tile.TileContext(nc) as tc, ExitStack() as es:
        cst_pool = es.enter_context(tc.tile_pool(name="const", bufs=1))
        wstage = es.enter_context(tc.tile_pool(name="wstage", bufs=1))
        xb_pool = es.enter_context(tc.tile_pool(name="xb", bufs=2))
        h_pool = es.enter_context(tc.tile_pool(name="hb", bufs=2))
        qk_pool = es.enter_context(tc.tile_pool(name="qk", bufs=8))
        vt_pool = es.enter_context(tc.tile_pool(name="vt", bufs=2))
        e_pool = es.enter_context(tc.tile_pool(name="ee", bufs=3))
        a_pool = es.enter_context(tc.tile_pool(name="ab", bufs=2))
        sm_pool = es.enter_context(tc.tile_pool(name="sm", bufs=2))
        rc_pool = es.enter_context(tc.tile_pool(name="rc", bufs=2))
        au_pool = es.enter_context(tc.tile_pool(name="au", bufs=2))
        bc_pool = es.enter_context(tc.tile_pool(name="bc", bufs=2))
        o_pool = es.enter_context(tc.tile_pool(name="ob", bufs=2))
        ps_s0 = es.enter_context(tc.tile_pool(name="ps_s0", bufs=1, space="PSUM"))
        ps_s1 = es.enter_context(tc.tile_pool(name="ps_s1", bufs=1, space="PSUM"))
        ps_aA = es.enter_context(tc.tile_pool(name="ps_aA", bufs=1, space="PSUM"))
        ps_aB = es.enter_context(tc.tile_pool(name="ps_aB", bufs=1, space="PSUM"))

        # ---- constants first on the sync queue (needed earliest), then
        # weights; x rides the gpsimd queue in parallel ----
        st_sel = wstage.tile([128, 4 * NG], F32, tag="selst")
        nc.sync.dma_start(st_sel, sel.ap())
        sel_sb = cst_pool.tile([128, 4 * NG], F32R, tag="sel")
        nc.vector.tensor_copy(sel_sb, st_sel)
        st_esel = wstage.tile([NG, 4 * 128], F32, tag="eselst")
        nc.sync.dma_start(st_esel, esel.ap())
        esel_sb = cst_pool.tile([NG, 4 * 128], F32R, tag="esel")
        nc.vector.tensor_copy(esel_sb, st_esel)
        cv = cst_pool.tile([128, 20], F32, tag="cvec")
        nc.sync.dma_start(cv, cvec.ap())
        wq_sb = []
        for j in range(4):
            wt = cst_pool.tile([128, 3 * C], BF16, tag=f"wq{j}")
            nc.sync.dma_start(wt, wqkT[128 * j : 128 * (j + 1), :])
            wq_sb.append(wt)
        wp_sb = []
        for j in range(4):
            wt = cst_pool.tile([128, C], BF16, tag=f"wp{j}")
            nc.sync.dma_start(wt, wpT[128 * j : 128 * (j + 1), :])
            wp_sb.append(wt)
        bq_sb = [cv[:, i : i + 1] for i in range(0, 4)]
        bk_sb = [cv[:, i : i + 1] for i in range(4, 8)]
        beff_sb = [cv[:, i : i + 1] for i in range(8, 12)]
        nw_sb = [cv[:, i : i + 1] for i in range(12, 16)]
        nb_sb = [cv[:, i : i + 1] for i in range(16, 20)]
        ones_b = cst_pool.tile([128, 1], BF16, tag="ones_b")
        nc.vector.memset(ones_b, 1.0)
        ones1b = cst_pool.tile([1, 64], BF16, tag="ones1b")
        nc.vector.memset(ones1b, 1.0)

        # contiguous slices into the host-reordered weight:
        # cols [0:512]=q pair-major, [512:1024]=k pair-major,
        # [1024:1536]=v head-major
        def wq_ap(j, p):
            return wq_sb[j][:, 128 * p : 128 * (p + 1)]

        def wk_ap(j, p):
            return wq_sb[j][:, 512 + 128 * p : 512 + 128 * (p + 1)]

        def emit_prep(b):
            """x load + groupnorm stats + normalized h (bf16)."""
            xt = []
            mv_t = []
            for j in range(4):
                x_t = xb_pool.tile([128, L], F32, tag=f"x{j}")
                nc.gpsimd.dma_start(x_t, x2[b, 128 * j : 128 * (j + 1), :])
                xt.append(x_t)
                stats = sm_pool.tile([128, 2, 6], F32, tag="bnst")
                nc.vector.bn_stats(stats[:, 0, :], x_t[:, 0:512])
                nc.vector.bn_stats(stats[:, 1, :], x_t[:, 512:1024])
                mv = sm_pool.tile([128, 2], F32, tag=f"mv{j}")
                nc.vector.bn_aggr(mv, stats)
                mv_t.append(mv)

            # ss = [mean, E[x^2]] per channel -> group reduce on PE
            gps = ps_s0.tile([128, 2], F32, tag="s0")
            for j in range(4):
                ss = sm_pool.tile([128, 2], F32R, tag=f"ss{j}")
                msq = sm_pool.tile([128, 1], F32, tag=f"msq{j}")
                nc.vector.tensor_copy(ss[:, 0:1], mv_t[j][:, 0:1])
                nc.vector.tensor_tensor(
                    msq, mv_t[j][:, 0:1], mv_t[j][:, 0:1], OP.mult
                )
                nc.vector.tensor_tensor(ss[:, 1:2], mv_t[j][:, 1:2], msq, OP.add)
                nc.tensor.matmul(
                    gps[0:NG, 0:2],
                    sel_sb[:, NG * j : NG * (j + 1)],
                    ss,
                    start=(j == 0),
                    stop=(j == 3),
                )

            # group stats -> [mean, rstd]
            gsb = sm_pool.tile([NG, 2], F32, tag="gsb")
            nc.vector.tensor_copy(gsb, gps[0:NG, 0:2])
            gm2 = sm_pool.tile([NG, 1], F32, tag="gm2")
            nc.vector.tensor_tensor(gm2, gsb[:, 0:1], gsb[:, 0:1], OP.mult)
            gvar = sm_pool.tile([NG, 1], F32, tag="gvar")
            nc.vector.tensor_tensor(gvar, gsb[:, 1:2], gm2, OP.subtract)
            nc.vector.tensor_scalar_add(gvar, gvar, EPS)
            # rsqrt via bit trick + 2 Newton iterations (no ACT table swap)
            y_i = sm_pool.tile([NG, 1], mybir.dt.int32, tag="rs_i")
            nc.vector.tensor_scalar(
                y_i, gvar.bitcast(mybir.dt.int32), 1, None, OP.logical_shift_right
            )
            nc.vector.tensor_scalar(y_i, y_i, -1, 0x5F3759DF, OP.mult, OP.add)
            y = y_i.bitcast(F32)
            gstats = sm_pool.tile([NG, 2], F32R, tag="gst")
            nc.vector.tensor_copy(gstats[:, 0:1], gsb[:, 0:1])
            tmp = sm_pool.tile([NG, 1], F32, tag="rs_t")
            for _ in range(2):
                nc.vector.tensor_tensor(tmp, y, y, OP.mult)
                nc.vector.tensor_tensor(tmp, tmp, gvar, OP.mult)
                nc.vector.tensor_scalar(tmp, tmp, -0.5, 1.5, OP.mult, OP.add)
                nc.vector.tensor_tensor(y, y, tmp, OP.mult)
            nc.vector.tensor_copy(gstats[:, 1:2], y)

            # expand per-group -> per-channel, normalize -> h (bf16)
            ht = []
            for j in range(4):
                cst_ps = ps_s1.tile([128, 2], F32, tag="s1")
                nc.tensor.matmul(
                    cst_ps[:, 0:2],
                    esel_sb[:, 128 * j : 128 * (j + 1)],
                    gstats,
                    start=True,
                    stop=True,
                )
                sc = sm_pool.tile([128, 1], F32, tag=f"sc{j}")
                nc.vector.tensor_tensor(sc, cst_ps[:, 1:2], nw_sb[j], OP.mult)
                bi = sm_pool.tile([128, 1], F32, tag=f"bi{j}")
                nc.vector.tensor_tensor(bi, cst_ps[:, 0:1], sc, OP.mult)
                nc.vector.tensor_tensor(bi, nb_sb[j], bi, OP.subtract)
                h_t = h_pool.tile([128, L], BF16, tag=f"h{j}")
                nc.vector.tensor_scalar(h_t, xt[j], sc, bi, OP.mult, OP.add)
                ht.append(h_t)
            return ht

        def emit_vt(ht):
            """v^T: per head 64 v-cols + ones column (softmax denominator)."""
            vt_sb = vt_pool.tile([128, 8, 8, 65], BF16, tag="vt")
            nc.vector.tensor_copy(
                vt_sb[:, :, :, 64:65],
                ones_b[:, None, None, :].to_broadcast((128, 8, 8, 1)),
            )
            for i in range(8):
                vpool, vtag = ((ps_s0, "s0"), (ps_s1, "s1"))[i % 2]
                vps = vpool.tile([128, 512], F32, tag=vtag)
                for j in range(4):
                    nc.tensor.matmul(
                        vps,
                        ht[j][:, 128 * i : 128 * (i + 1)],
                        wq_sb[j][:, 1024:1536],
                        start=(j == 0),
                        stop=(j == 3),
                    )
                nc.vector.tensor_copy(
                    vt_sb[:, i, :, 0:64],
                    vps[:, 0:512].rearrange("p (h c) -> p h c", c=64),
                )
            return vt_sb

        def emit_qkv(ht):
            """q/k for all 4 pairs -> bf16 SBUF tiles."""
            qk_t = []
            for p in range(4):
                qps = ps_s0.tile([128, 1024], F32, tag="s0")
                kps = ps_s1.tile([128, 1024], F32, tag="s1")
                for n in range(2):
                    for j in range(4):
                        nc.tensor.matmul(
                            qps[:, 512 * n : 512 * (n + 1)],
                            wq_ap(j, p),
                            ht[j][:, 512 * n : 512 * (n + 1)],
                            start=(j == 0),
                            stop=(j == 3),
                        )
                        nc.tensor.matmul(
                            kps[:, 512 * n : 512 * (n + 1)],
                            wk_ap(j, p),
                            ht[j][:, 512 * n : 512 * (n + 1)],
                            start=(j == 0),
                            stop=(j == 3),
                        )
                q_sb = qk_pool.tile([128, L], BF16, tag="q")
                nc.vector.tensor_scalar_add(q_sb, qps, bq_sb[p])
                k_sb = qk_pool.tile([128, L], BF16, tag="k")
                nc.vector.tensor_scalar_add(k_sb, kps, bk_sb[p])
                qk_t.append((q_sb, k_sb))
            return qk_t

        def emit_attention_pair(p, q_sb, k_sb, vt_sb):
            """S -> exp -> PV for head pair p; returns normalized a (bf16)."""
            a_psA = ps_aA.tile([128, 1024], F32, tag="aA")
            a_psB = ps_aB.tile([128, 1024], F32, tag="aB")
            for sj in range(8):
                sl = slice(128 * sj, 128 * (sj + 1))
                s_ps0 = ps_s0.tile([128, 1024], F32, tag="s0")
                s_ps1 = ps_s1.tile([128, 1024], F32, tag="s1")
                for n in range(2):
                    nc.tensor.matmul(
                        s_ps0[:, 512 * n : 512 * (n + 1)],
                        k_sb[0:64, sl],
                        q_sb[0:64, 512 * n : 512 * (n + 1)],
                        start=True,
                        stop=True,
                        tile_position=(0, 0),
                    )
                    nc.tensor.matmul(
                        s_ps1[:, 512 * n : 512 * (n + 1)],
                        k_sb[64:128, sl],
                        q_sb[64:128, 512 * n : 512 * (n + 1)],
                        start=True,
                        stop=True,
                        tile_position=(64, 0),
                    )
                for half, (s_ps, a_ps) in enumerate(
                    ((s_ps0, a_psA), (s_ps1, a_psB))
                ):
                    e_t = e_pool.tile([128, 1024], BF16, tag="e")
                    nc.scalar.activation(e_t, s_ps, AF.Exp, scale=0.125)
                    for n in range(2):
                        nsl = slice(512 * n, 512 * (n + 1))
                        nc.tensor.matmul(
                            a_ps[0:65, nsl],
                            vt_sb[:, sj, 2 * p + half, :],
                            e_t[:, nsl],
                            start=(sj == 0),
                            stop=(sj == 7),
                        )

            # denominators -> 1/d (approx) -> col-tiled ones-matmul
            # broadcast (both halves concurrent in one psum tile)
            a_t = a_pool.tile([128, L], BF16, tag=f"a{p}")
            rcs = []
            for half, a_ps in ((0, a_psA), (1, a_psB)):
                dsb = rc_pool.tile([1, 1024], F32, tag="dsb")
                if half == 0:
                    nc.scalar.activation(dsb, a_ps[64:65, :], AF.Copy)
                else:
                    nc.vector.tensor_copy(dsb, a_ps[64:65, :])
                raw = rc_pool.tile([1, 1024], F32, tag="raw")
                nc.vector.reciprocal_approx_fast(raw, dsb)
                rc_b = rc_pool.tile([1, 1024], BF16, tag="rc")
                nc.vector.tensor_copy(rc_b, raw)
                rcs.append(rc_b)
            a_unA = au_pool.tile([64, 1024], BF16, tag="a_un")
            nc.vector.tensor_copy(a_unA, a_psA[0:64, :])
            a_unB = au_pool.tile([64, 1024], BF16, tag="a_un")
            nc.vector.tensor_copy(a_unB, a_psB[0:64, :])
            bc_ps = ps_aA.tile([128, 1024], F32, tag="aA")
            for n in range(2):
                nsl = slice(512 * n, 512 * (n + 1))
                nc.tensor.matmul(
                    bc_ps[0:64, nsl], ones1b, rcs[0][:, nsl],
                    start=True, stop=True, tile_position=(0, 0),
                )
                nc.tensor.matmul(
                    bc_ps[64:128, nsl], ones1b, rcs[1][:, nsl],
                    start=True, stop=True, tile_position=(0, 64),
                )
            nc.vector.tensor_tensor(a_t[0:64, :], a_unA, bc_ps[0:64, :], OP.mult)
            nc.vector.tensor_tensor(
                a_t[64:128, :], a_unB, bc_ps[64:128, :], OP.mult
            )
            return a_t

        def emit_proj(bb, ab_tiles):
            for m in range(4):
                pj_pool, pj_tag = ((ps_aA, "aA"), (ps_aB, "aB"))[m % 2]
                pj = pj_pool.tile([128, 1024], F32, tag=pj_tag)
                for n in range(2):
                    for j in range(4):
                        nc.tensor.matmul(
                            pj[:, 512 * n : 512 * (n + 1)],
                            wp_sb[j][:, 128 * m : 128 * (m + 1)],
                            ab_tiles[j][:, 512 * n : 512 * (n + 1)],
                            start=(j == 0),
                            stop=(j == 3),
                        )
                o_t = o_pool.tile([128, L], F32, tag="o")
                nc.vector.tensor_scalar_add(o_t, pj, beff_sb[m])
                nc.sync.dma_start(out_d[bb, 128 * m : 128 * (m + 1), :], o_t)

        # ---- software-pipelined main loop: image b+1's prep/vt/qkv are
        # emitted inside image b's attention so every engine always has
        # ready work (PE density also keeps the clock un-throttled) ----
        ht = emit_prep(0)
        vt_sb = emit_vt(ht)
        qk_t = emit_qkv(ht)
        for b in range(BPC):
            a_sb = []
            ht_n = vt_n = qk_n = None
            for p in range(4):
                a_sb.append(
                    emit_attention_pair(p, qk_t[p][0], qk_t[p][1], vt_sb)
                )
                if b + 1 < BPC:
                    if p == 0:
                        ht_n = emit_prep(b + 1)
                    elif p == 1:
                        vt_n = emit_vt(ht_n)
                    elif p == 2:
                        qk_n = emit_qkv(ht_n)
            emit_proj(b, a_sb)
            if b + 1 < BPC:
                vt_sb, qk_t = vt_n, qk_n

    nc.compile()
    return nc


def _get_nc():
    global _nc_cache
    if _nc_cache is None:
        _nc_cache = _build()
    return _nc_cache


def _prep_inputs(x, norm_w, norm_b, w_qkv, b_qkv, w_proj, b_proj):
    import ml_dtypes

    x = np.asarray(x, dtype=np.float32).reshape(B, C, L)
    w_qkv = np.asarray(w_qkv, dtype=np.float32)
    b_qkv = np.asarray(b_qkv, dtype=np.float32)
    w_proj = np.asarray(w_proj, dtype=np.float32)
    b_proj = np.asarray(b_proj, dtype=np.float32)
    norm_w = np.asarray(norm_w, dtype=np.float32)
    norm_b = np.asarray(norm_b, dtype=np.float32)

    # column-reordered transposed qkv weight: [C, 3C] with
    # q pair-major | k pair-major | v head-major, all contiguous
    wqkT = np.zeros((C, 3 * C), dtype=np.float32)
    wT = w_qkv.T  # [C, 3C] original row order (per head: q,k,v)
    for h in range(NH):
        base = 192 * h
        wqkT[:, 64 * h : 64 * (h + 1)] = wT[:, base : base + 64]
        wqkT[:, 512 + 64 * h : 512 + 64 * (h + 1)] = wT[:, base + 64 : base + 128]
        wqkT[:, 1024 + 64 * h : 1024 + 64 * (h + 1)] = wT[:, base + 128 : base + 192]
    wqkT = np.ascontiguousarray(wqkT.astype(ml_dtypes.bfloat16))
    wpT = np.ascontiguousarray(w_proj.T.astype(ml_dtypes.bfloat16))  # [C, C]

    # per-pair q/k biases: [pair, {q,k}, 128]
    bqk = np.zeros((4, 2, 128), dtype=np.float32)
    for p in range(4):
        for half, h in enumerate((2 * p, 2 * p + 1)):
            base = 192 * h
            bqk[p, 0, 64 * half : 64 * (half + 1)] = b_qkv[base : base + 64]
            bqk[p, 1, 64 * half : 64 * (half + 1)] = b_qkv[base + 64 : base + 128]
    del base

    # v bias folded into proj bias: b_eff = b_proj + w_proj @ bv
    bv = np.zeros((C,), dtype=np.float32)
    for h in range(NH):
        bv[64 * h : 64 * (h + 1)] = b_qkv[192 * h + 128 : 192 * h + 192]
    b_eff = (b_proj.astype(np.float64) + w_proj.astype(np.float64) @ bv).astype(
        np.float32
    )

    sel = np.zeros((128, 4 * NG), dtype=np.float32)
    esel = np.zeros((NG, 4 * 128), dtype=np.float32)
    for j in range(4):
        for c in range(128):
            sel[c, NG * j + 8 * j + c // GS] = 1.0 / GS
            esel[8 * j + c // GS, 128 * j + c] = 1.0

    cv = np.zeros((128, 20), dtype=np.float32)
    cv[:, 0:4] = bqk[:, 0, :].T
    cv[:, 4:8] = bqk[:, 1, :].T
    cv[:, 8:12] = b_eff.reshape(4, 128).T
    cv[:, 12:16] = norm_w.reshape(4, 128).T
    cv[:, 16:20] = norm_b.reshape(4, 128).T

    shared = {
        "wqkT": wqkT,
        "wpT": wpT,
        "cvec": cv,
        "sel": sel,
        "esel": esel,
    }
    in_maps = []
    for c in range(N_CORES):
        m = dict(shared)
        m["x2"] = np.ascontiguousarray(x[BPC * c : BPC * (c + 1)])
        in_maps.append(m)
    return in_maps


def _run(in_maps, trace=False):
    nc = _get_nc()
    return run_bass_kernel_spmd(
        nc, in_maps, core_ids=list(range(N_CORES)), trace=trace
    )


def kernel(x, norm_w, norm_b, w_qkv, b_qkv, w_proj, b_proj):
    in_maps = _prep_inputs(x, norm_w, norm_b, w_qkv, b_qkv, w_proj, b_proj)
    res = _run(in_maps)
    out = np.concatenate([r["out"] for r in res.results], axis=0)
    return out.astype(np.float32)
